# revision 1
# baseline (speedup 1.0000x reference)
"""HGRN attention Trainium2 kernel.

Sharding: B*L (4 batches x 4096 tokens) split into 8 chunks of T=2048 tokens,
one per NeuronCore: core c = 2*b + half handles tokens [half*T, (half+1)*T) of
batch b. The gated linear recurrence h_t = sigmoid(f_t)*h_{t-1} + swiglu-input
runs per (batch, channel); the cross-chunk carry (h at the half boundary) is
exchanged with a tiny pairwise AllReduce and applied as h_local + cumprod*carry
(cumprod underflows to exactly 0 in fp32 past ~130 steps, so only the first 256
columns of each odd chunk need the fixup - bit-matching the fp32 reference).

On-chip layout is transposed ([channel, time]) so the recurrence maps onto the
DVE tensor_tensor_scan instruction; the host pre-transposes x and the weights,
and the kernel emits y transposed (host transposes back). Matmuls run in
float32r (tf32-rate, ~1.5e-4 rel err). RMSNorm over channels uses a full
ONES[128x128] matmul for the cross-partition reduce+broadcast.
"""
import numpy as np

import concourse.bacc as bacc
import concourse.tile as tile
import concourse.mybir as mybir
from concourse.bass_utils import run_bass_kernel_spmd

B, L, D = 4, 4096, 2048
T = 2048                 # tokens per core
NCORE = 8
ET = DT = D // 128       # 16 tiles of 128 channels
TB1 = 1024               # phase-1 time block
NB1 = T // TB1
TB2 = 512                # phase-2/3 time block
NB2 = T // TB2
CLEN = 256               # cumprod fixup length (0 in fp32 beyond this)
EPS = 1e-5

F32 = mybir.dt.float32
F32R = mybir.dt.float32r
AF = mybir.ActivationFunctionType
OP = mybir.AluOpType

_CACHE = {}


def _build():
    nc = bacc.Bacc("TRN2", target_bir_lowering=False, debug=False,
                   enable_asserts=True, num_devices=NCORE)
    xt_d = nc.dram_tensor("xt", [D, T], F32R, kind="ExternalInput")
    wi_d = nc.dram_tensor("wi", [D, D], F32R, kind="ExternalInput")   # (d_in, e)
    wf_d = nc.dram_tensor("wf", [D, D], F32R, kind="ExternalInput")
    wg_d = nc.dram_tensor("wg", [D, D], F32R, kind="ExternalInput")
    wo_d = nc.dram_tensor("wo", [D, D], F32R, kind="ExternalInput")   # (e, d_out)
    gnw_d = nc.dram_tensor("gnw", [128, ET], F32, kind="ExternalInput")
    mask_d = nc.dram_tensor("mask", [128, 1], F32, kind="ExternalInput")
    yt_d = nc.dram_tensor("yt", [D, T], F32, kind="ExternalOutput")

    with tile.TileContext(nc) as tc:
        with tc.tile_pool(name="persist", bufs=1) as pp, \
             tc.tile_pool(name="dram", bufs=1, space="DRAM") as dr:
            carry = pp.tile([128, ET], F32, tag="carry")
            recv = pp.tile([128, ET], F32, tag="recv")
            cin = pp.tile([128, ET], F32, tag="cin")
            gnw = pp.tile([128, ET], F32, tag="gnw")
            maskt = pp.tile([128, 1], F32, tag="mask")
            acc = pp.tile([128, T], F32, tag="acc")
            call = pp.tile([128, ET * CLEN], F32, tag="call")
            rms = pp.tile([128, T], F32, tag="rms")
            ones = pp.tile([128, 128], F32, tag="ones")

            h_sp = dr.tile([D, T], F32, tag="hsp")
            g_sp = dr.tile([D, T], F32, tag="gsp")
            hl_i = dr.tile([128, ET], F32, tag="hli")
            hl_o = dr.tile([128, ET], F32, tag="hlo")

            nc.vector.memset(carry[:], 0.0)
            nc.vector.memset(ones[:], 1.0)
            nc.sync.dma_start(gnw[:], gnw_d.ap()[:])
            nc.sync.dma_start(maskt[:], mask_d.ap()[:])

            # ---------------- phase 1: projections + scan + spills ----------
            with tc.tile_pool(name="xtp", bufs=1) as xtp, \
                 tc.tile_pool(name="wp", bufs=2) as wp, \
                 tc.tile_pool(name="wk", bufs=2) as wk, \
                 tc.tile_pool(name="pj", bufs=1, space="PSUM") as pj:
                for tb in range(NB1):
                    ts0 = tb * TB1
                    xt = xtp.tile([128, DT * TB1], F32R, tag="xt")
                    for dt in range(DT):
                        nc.sync.dma_start(
                            xt[:, dt * TB1:(dt + 1) * TB1],
                            xt_d.ap()[dt * 128:(dt + 1) * 128, ts0:ts0 + TB1])
                    nc.vector.memset(acc[:, ts0:ts0 + TB1], 0.0)
                    for et in range(ET):
                        es = slice(et * 128, (et + 1) * 128)
                        wts = []
                        for nm, wd in (("wi", wi_d), ("wf", wf_d), ("wg", wg_d)):
                            w = wp.tile([128, DT * 128], F32R, tag=nm)
                            nc.sync.dma_start(
                                w[:].rearrange("p (dt e) -> p dt e", e=128),
                                wd.ap().rearrange("(dt p) e -> p dt e",
                                                  p=128)[:, :, es])
                            wts.append(w)
                        ps = {}
                        for nm, w in zip(("i", "f", "g"), wts):
                            p = pj.tile([128, TB1], F32, tag="p" + nm)
                            for n in range(TB1 // 512):
                                for dt in range(DT):
                                    nc.tensor.matmul(
                                        p[:, n * 512:(n + 1) * 512],
                                        w[:, dt * 128:(dt + 1) * 128],
                                        xt[:, dt * TB1 + n * 512:
                                           dt * TB1 + (n + 1) * 512],
                                        start=(dt == 0), stop=(dt == DT - 1))
                            ps[nm] = p
                        gate = wk.tile([128, TB1], F32, tag="gate")
                        nc.scalar.activation(gate[:], ps["f"][:], AF.Sigmoid)
                        sil = wk.tile([128, TB1], F32, tag="sil")
                        nc.scalar.activation(sil[:], ps["i"][:], AF.Silu)
                        omg = wk.tile([128, TB1], F32, tag="omg")
                        nc.vector.tensor_scalar(omg[:], gate[:], -1.0, 1.0,
                                                OP.mult, OP.add)
                        iin = wk.tile([128, TB1], F32, tag="iin")
                        nc.vector.tensor_mul(iin[:], omg[:], sil[:])
                        h1 = wk.tile([128, TB1], F32, tag="h1")
                        nc.vector.tensor_tensor_scan(
                            h1[:], gate[:], iin[:], carry[:, et:et + 1],
                            OP.mult, OP.add)
                        nc.vector.tensor_copy(carry[:, et:et + 1],
                                              h1[:, TB1 - 1:TB1])
                        if tb == 0:
                            nc.vector.tensor_tensor_scan(
                                call[:, et * CLEN:(et + 1) * CLEN],
                                gate[:, 0:CLEN], gate[:, 0:CLEN], 1.0,
                                OP.mult, OP.bypass)
                        g1 = wk.tile([128, TB1], F32, tag="g1")
                        nc.scalar.copy(g1[:], ps["g"][:])
                        sq = wk.tile([128, TB1], F32, tag="sq")
                        nc.scalar.activation(sq[:], ps["g"][:], AF.Square)
                        nc.vector.tensor_add(acc[:, ts0:ts0 + TB1],
                                             acc[:, ts0:ts0 + TB1], sq[:])
                        nc.sync.dma_start(
                            h_sp[et * 128:(et + 1) * 128, ts0:ts0 + TB1], h1[:])
                        nc.sync.dma_start(
                            g_sp[et * 128:(et + 1) * 128, ts0:ts0 + TB1], g1[:])

            # ---------------- phase 1.5: carry exchange + rmsnorm -----------
            nc.sync.dma_start(hl_i[:], carry[:])
            nc.gpsimd.collective_compute(
                "AllReduce", OP.add,
                replica_groups=[[0, 1], [2, 3], [4, 5], [6, 7]],
                ins=[hl_i.opt()], outs=[hl_o.opt()])
            nc.sync.dma_start(recv[:], hl_o[:])
            nc.vector.tensor_sub(recv[:], recv[:], carry[:])
            nc.vector.tensor_scalar(cin[:], recv[:], maskt[:, 0:1], None,
                                    OP.mult)

            with tc.tile_pool(name="sp", bufs=1, space="PSUM") as sp, \
                 tc.tile_pool(name="rwk", bufs=1) as rwk:
                S = sp.tile([128, T], F32, tag="S")
                for n in range(T // 512):
                    nc.tensor.matmul(S[:, n * 512:(n + 1) * 512], ones[:],
                                     acc[:, n * 512:(n + 1) * 512],
                                     start=True, stop=True)
                m = rwk.tile([128, T], F32, tag="m")
                nc.vector.tensor_scalar(m[:], S[:], 1.0 / D, EPS,
                                        OP.mult, OP.add)
                rec = rwk.tile([128, T], F32, tag="rec")
                nc.vector.reciprocal(rec[:], m[:])
                nc.scalar.activation(rms[:], rec[:], AF.Sqrt)

            # ---------------- phase 2+3: gating + output projection ---------
            with tc.tile_pool(name="op2", bufs=2) as op2, \
                 tc.tile_pool(name="outp", bufs=2) as outp, \
                 tc.tile_pool(name="wop", bufs=2) as wop, \
                 tc.tile_pool(name="yp", bufs=2, space="PSUM") as yp, \
                 tc.tile_pool(name="yo", bufs=2) as yo:
                for tb2 in range(NB2):
                    ts = tb2 * TB2
                    osb = outp.tile([128, ET * TB2], F32R, tag="osb")
                    for et in range(ET):
                        h2 = op2.tile([128, TB2], F32, tag="h2")
                        nc.sync.dma_start(
                            h2[:], h_sp[et * 128:(et + 1) * 128, ts:ts + TB2])
                        g2 = op2.tile([128, TB2], F32, tag="g2")
                        nc.sync.dma_start(
                            g2[:], g_sp[et * 128:(et + 1) * 128, ts:ts + TB2])
                        if tb2 == 0:
                            nc.vector.scalar_tensor_tensor(
                                h2[:, 0:CLEN],
                                call[:, et * CLEN:(et + 1) * CLEN],
                                cin[:, et:et + 1], h2[:, 0:CLEN],
                                OP.mult, OP.add)
                        sw = op2.tile([128, TB2], F32, tag="sw")
                        nc.scalar.activation(sw[:], h2[:], AF.Silu)
                        w1 = op2.tile([128, TB2], F32, tag="w1")
                        nc.vector.tensor_mul(w1[:], g2[:], rms[:, ts:ts + TB2])
                        nc.vector.scalar_tensor_tensor(
                            osb[:, et * TB2:(et + 1) * TB2], w1[:],
                            gnw[:, et:et + 1], sw[:], OP.mult, OP.mult)
                    for dt in range(DT):
                        wo = wop.tile([128, ET * 128], F32R, tag="wo")
                        nc.sync.dma_start(
                            wo[:].rearrange("p (et d) -> p et d", d=128),
                            wo_d.ap().rearrange("(et p) d -> p et d",
                                                p=128)[:, :, dt * 128:(dt + 1) * 128])
                        ypt = yp.tile([128, TB2], F32, tag="ypt")
                        for et in range(ET):
                            nc.tensor.matmul(
                                ypt[:], wo[:, et * 128:(et + 1) * 128],
                                osb[:, et * TB2:(et + 1) * TB2],
                                start=(et == 0), stop=(et == ET - 1))
                        ysb = yo.tile([128, TB2], F32, tag="ysb")
                        nc.scalar.copy(ysb[:], ypt[:])
                        nc.sync.dma_start(
                            yt_d.ap()[dt * 128:(dt + 1) * 128, ts:ts + TB2],
                            ysb[:])
    nc.compile()
    return nc


def _get_nc():
    if "nc" not in _CACHE:
        _CACHE["nc"] = _build()
    return _CACHE["nc"]


def kernel(hidden_states, Wi, Wf, Wg, g_norm_weight, Wo, **_unused):
    nc = _get_nc()
    wiT = np.ascontiguousarray(Wi.T)
    wfT = np.ascontiguousarray(Wf.T)
    wgT = np.ascontiguousarray(Wg.T)
    woT = np.ascontiguousarray(Wo.T)
    gnw = np.ascontiguousarray(
        np.asarray(g_norm_weight, np.float32).reshape(ET, 128).T)
    in_maps = []
    for c in range(NCORE):
        b, half = c // 2, c % 2
        xt = np.ascontiguousarray(
            hidden_states[b, half * T:(half + 1) * T, :].T)
        mask = np.full((128, 1), float(half), np.float32)
        in_maps.append({"xt": xt, "wi": wiT, "wf": wfT, "wg": wgT,
                        "wo": woT, "gnw": gnw, "mask": mask})
    res = run_bass_kernel_spmd(nc, in_maps, list(range(NCORE))).results
    y = np.empty((B, L, D), np.float32)
    for c in range(NCORE):
        b, half = c // 2, c % 2
        y[b, half * T:(half + 1) * T, :] = res[c]["yt"].T
    return y



# revision 3
# speedup vs baseline: 1.4466x; 1.4466x over previous
"""HGRN attention Trainium2 kernel (v2, bf16 data path).

Sharding: B*L (4 batches x 4096 tokens) split into 8 chunks of T=2048 tokens,
one per NeuronCore: core c = 2*b + half handles tokens [half*T, (half+1)*T) of
batch b. The gated linear recurrence h_t = sigmoid(f_t)*h_{t-1} + swiglu-input
runs per (batch, channel); the cross-chunk carry (h at the half boundary) is
exchanged with a tiny pairwise AllReduce and applied as h_local + cumprod*carry
(cumprod underflows to exactly 0 in fp32 past ~130 steps, so only the first 256
columns of each odd chunk need the fixup).

v2 layout: everything bf16 on the wire and in SBUF (rel err ~4e-3 vs the 2e-2
gate). Phase 1 streams x per 512-token quarter (resident in SBUF), loops output
tiles et inside, and keeps the gated output p = g*gnw*silu(h) entirely in SBUF
(8 MiB bf16) - no DRAM spills. PSUM is double-buffered at 512-wide tiles so the
PE never waits on consumers. Phase 2 holds all of Wo resident (8 MiB bf16) and
processes time blocks in order (1,2,3,0) so the carry AllReduce latency hides
under ~160us of o_proj compute; only tokens [0,256) wait for it. Host
pre-packs weights so every DMA is a contiguous [128, 2048] block.
"""
import numpy as np
import ml_dtypes

import concourse.bacc as bacc
import concourse.tile as tile
import concourse.mybir as mybir
from concourse.bass_utils import run_bass_kernel_spmd

B, L, D = 4, 4096, 2048
T = 2048                 # tokens per core
NCORE = 8
ET = DT = D // 128       # 16 tiles of 128 channels
TQ = 512                 # time block (quarter of T)
NQ = T // TQ
CLEN = 256               # cumprod fixup length (0 in fp32 beyond this)
EPS = 1e-5

F32 = mybir.dt.float32
F32R = mybir.dt.float32r
BF16 = mybir.dt.bfloat16
AF = mybir.ActivationFunctionType
OP = mybir.AluOpType

_CACHE = {}


def _build():
    nc = bacc.Bacc("TRN2", target_bir_lowering=False, debug=False,
                   enable_asserts=True, num_devices=NCORE)
    xt_d = nc.dram_tensor("xt", [D, T], BF16, kind="ExternalInput")
    wi_d = nc.dram_tensor("wi", [ET * 128, DT * 128], BF16, kind="ExternalInput")
    wf_d = nc.dram_tensor("wf", [ET * 128, DT * 128], BF16, kind="ExternalInput")
    wg_d = nc.dram_tensor("wg", [ET * 128, DT * 128], BF16, kind="ExternalInput")
    wo_d = nc.dram_tensor("wo", [DT * 128, ET * 128], BF16, kind="ExternalInput")
    gnw_d = nc.dram_tensor("gnw", [128, ET], F32, kind="ExternalInput")
    mask_d = nc.dram_tensor("mask", [128, 1], F32, kind="ExternalInput")
    yt_d = nc.dram_tensor("yt", [D, T], F32, kind="ExternalOutput")

    with tile.TileContext(nc) as tc:
        with tc.tile_pool(name="persist", bufs=1) as pp, \
             tc.tile_pool(name="dram", bufs=1, space="DRAM") as dr:
            carry = pp.tile([128, ET], F32, tag="carry")
            recv = pp.tile([128, ET], F32, tag="recv")
            cin = pp.tile([128, ET], F32, tag="cin")
            gnw = pp.tile([128, ET], F32, tag="gnw")
            maskt = pp.tile([128, 1], F32, tag="mask")
            rms = pp.tile([128, T], F32, tag="rms")
            call = pp.tile([128, ET * CLEN], BF16, tag="call")
            hkeep = pp.tile([128, ET * CLEN], BF16, tag="hkeep")
            gkeep = pp.tile([128, ET * CLEN], BF16, tag="gkeep")
            ones = pp.tile([128, 128], F32, tag="ones")
            psb = pp.tile([128, ET * T], BF16, tag="psb")

            hl_i = dr.tile([128, ET], F32, tag="hli")
            hl_o = dr.tile([128, ET], F32, tag="hlo")

            nc.vector.memset(carry[:], 0.0)
            nc.vector.memset(ones[:], 1.0)
            nc.sync.dma_start(gnw[:], gnw_d.ap()[:])
            nc.sync.dma_start(maskt[:], mask_d.ap()[:])

            # ---------------- phase 1: projections + scan, p kept in SBUF ---
            with tc.tile_pool(name="accp", bufs=1) as ap_:
                acc = ap_.tile([128, T], F32, tag="acc")
                with tc.tile_pool(name="xq", bufs=2) as xqp, \
                     tc.tile_pool(name="wp", bufs=2) as wp, \
                     tc.tile_pool(name="wk", bufs=2) as wk, \
                     tc.tile_pool(name="pj", bufs=2, space="PSUM") as pj:
                    for q in range(NQ):
                        ts0 = q * TQ
                        # weights for the first et land before x on q==0 so the
                        # PE starts as early as possible
                        pre_ws = None
                        if q == 0:
                            pre_ws = {}
                            for nm, wd in (("wf", wf_d), ("wi", wi_d),
                                           ("wg", wg_d)):
                                w = wp.tile([128, DT * 128], BF16, tag=nm)
                                nc.sync.dma_start(w[:], wd.ap()[0:128, :])
                                pre_ws[nm] = w
                        xq = xqp.tile([128, DT * TQ], BF16, tag="xq")
                        for dt in range(DT):
                            nc.sync.dma_start(
                                xq[:, dt * TQ:(dt + 1) * TQ],
                                xt_d.ap()[dt * 128:(dt + 1) * 128,
                                          ts0:ts0 + TQ])
                        nc.vector.memset(acc[:, ts0:ts0 + TQ], 0.0)
                        for et in range(ET):
                            if q == 0 and et == 0:
                                ws = pre_ws
                            else:
                                ws = {}
                                for nm, wd in (("wf", wf_d), ("wi", wi_d),
                                               ("wg", wg_d)):
                                    w = wp.tile([128, DT * 128], BF16, tag=nm)
                                    nc.sync.dma_start(
                                        w[:],
                                        wd.ap()[et * 128:(et + 1) * 128, :])
                                    ws[nm] = w
                            ps = {}
                            for nm in ("pf", "pi", "pg"):
                                p = pj.tile([128, TQ], F32, tag=nm)
                                w = ws["w" + nm[1]]
                                for dt in range(DT):
                                    nc.tensor.matmul(
                                        p[:], w[:, dt * 128:(dt + 1) * 128],
                                        xq[:, dt * TQ:(dt + 1) * TQ],
                                        start=(dt == 0), stop=(dt == DT - 1))
                                ps[nm] = p
                            gate = wk.tile([128, TQ], F32, tag="gate")
                            nc.scalar.activation(gate[:], ps["pf"][:],
                                                 AF.Sigmoid)
                            sil = wk.tile([128, TQ], F32, tag="sil")
                            nc.scalar.activation(sil[:], ps["pi"][:], AF.Silu)
                            sq = wk.tile([128, TQ], F32, tag="sq")
                            nc.scalar.activation(sq[:], ps["pg"][:], AF.Square)
                            omg = wk.tile([128, TQ], F32, tag="omg")
                            nc.vector.tensor_scalar(omg[:], gate[:], -1.0, 1.0,
                                                    OP.mult, OP.add)
                            iin = wk.tile([128, TQ], F32, tag="iin")
                            nc.vector.tensor_mul(iin[:], omg[:], sil[:])
                            h1 = wk.tile([128, TQ], F32, tag="h1")
                            nc.vector.tensor_tensor_scan(
                                h1[:], gate[:], iin[:], carry[:, et:et + 1],
                                OP.mult, OP.add)
                            nc.vector.tensor_copy(carry[:, et:et + 1],
                                                  h1[:, TQ - 1:TQ])
                            if q == 0:
                                nc.vector.tensor_tensor_scan(
                                    call[:, et * CLEN:(et + 1) * CLEN],
                                    gate[:, 0:CLEN], gate[:, 0:CLEN], 1.0,
                                    OP.mult, OP.bypass)
                                nc.scalar.copy(
                                    hkeep[:, et * CLEN:(et + 1) * CLEN],
                                    h1[:, 0:CLEN])
                                nc.scalar.copy(
                                    gkeep[:, et * CLEN:(et + 1) * CLEN],
                                    ps["pg"][:, 0:CLEN])
                            nc.vector.tensor_add(acc[:, ts0:ts0 + TQ],
                                                 acc[:, ts0:ts0 + TQ], sq[:])
                            sw = wk.tile([128, TQ], F32, tag="sw")
                            nc.scalar.activation(sw[:], h1[:], AF.Silu)
                            nc.vector.scalar_tensor_tensor(
                                psb[:, et * T + ts0:et * T + ts0 + TQ],
                                ps["pg"][:], gnw[:, et:et + 1], sw[:],
                                OP.mult, OP.mult)

                # ------------- phase 1.5: carry exchange + rmsnorm ----------
                nc.sync.dma_start(hl_i[:], carry[:])
                nc.gpsimd.collective_compute(
                    "AllReduce", OP.add,
                    replica_groups=[[0, 1], [2, 3], [4, 5], [6, 7]],
                    ins=[hl_i.opt()], outs=[hl_o.opt()])
                nc.sync.dma_start(recv[:], hl_o[:])

                with tc.tile_pool(name="sp", bufs=2, space="PSUM") as sp, \
                     tc.tile_pool(name="rwk", bufs=2) as rwk:
                    for tb2 in (1, 2, 3, 0):
                        ts = tb2 * TQ
                        S = sp.tile([128, TQ], F32, tag="S")
                        nc.tensor.matmul(S[:], ones[:], acc[:, ts:ts + TQ],
                                         start=True, stop=True)
                        m = rwk.tile([128, TQ], F32, tag="m")
                        nc.vector.tensor_scalar(m[:], S[:], 1.0 / D, EPS,
                                                OP.mult, OP.add)
                        rec = rwk.tile([128, TQ], F32, tag="rec")
                        nc.vector.reciprocal(rec[:], m[:])
                        nc.scalar.activation(rms[:, ts:ts + TQ], rec[:],
                                             AF.Sqrt)

            # ---------------- phase 2: gating + output projection -----------
            with tc.tile_pool(name="wop", bufs=1) as wop, \
                 tc.tile_pool(name="osp", bufs=2) as osp, \
                 tc.tile_pool(name="fxp", bufs=2) as fxp, \
                 tc.tile_pool(name="yp", bufs=4, space="PSUM") as yp, \
                 tc.tile_pool(name="yo", bufs=2) as yo:
                wo = wop.tile([128, DT * ET * 128], BF16, tag="wo")
                for dt in range(DT):
                    nc.sync.dma_start(
                        wo[:, dt * ET * 128:(dt + 1) * ET * 128],
                        wo_d.ap()[dt * 128:(dt + 1) * 128, :])
                for tb2 in (1, 2, 3, 0):
                    ts = tb2 * TQ
                    if tb2 == 0:
                        # collective has finished long ago; apply the carry
                        # fixup to the first CLEN tokens of each channel tile
                        nc.vector.tensor_sub(recv[:], recv[:], carry[:])
                        nc.vector.tensor_scalar(cin[:], recv[:],
                                                maskt[:, 0:1], None, OP.mult)
                        for et in range(ET):
                            cs = slice(et * CLEN, (et + 1) * CLEN)
                            hfx = fxp.tile([128, CLEN], F32, tag="hfx")
                            nc.vector.scalar_tensor_tensor(
                                hfx[:], call[:, cs], cin[:, et:et + 1],
                                hkeep[:, cs], OP.mult, OP.add)
                            swf = fxp.tile([128, CLEN], F32, tag="swf")
                            nc.scalar.activation(swf[:], hfx[:], AF.Silu)
                            nc.vector.scalar_tensor_tensor(
                                psb[:, et * T:et * T + CLEN],
                                gkeep[:, cs], gnw[:, et:et + 1], swf[:],
                                OP.mult, OP.mult)
                    ot = osp.tile([128, ET * TQ], BF16, tag="osb")
                    for et in range(ET):
                        nc.vector.tensor_mul(
                            ot[:, et * TQ:(et + 1) * TQ],
                            psb[:, et * T + ts:et * T + ts + TQ],
                            rms[:, ts:ts + TQ])
                    for dt in range(DT):
                        ypt = yp.tile([128, TQ], F32, tag="ypt")
                        for et in range(ET):
                            nc.tensor.matmul(
                                ypt[:],
                                wo[:, (dt * ET + et) * 128:
                                   (dt * ET + et + 1) * 128],
                                ot[:, et * TQ:(et + 1) * TQ],
                                start=(et == 0), stop=(et == ET - 1))
                        ysb = yo.tile([128, TQ], F32, tag="ysb")
                        nc.scalar.copy(ysb[:], ypt[:])
                        nc.sync.dma_start(
                            yt_d.ap()[dt * 128:(dt + 1) * 128, ts:ts + TQ],
                            ysb[:])
    nc.compile()
    return nc


def _get_nc():
    if "nc" not in _CACHE:
        _CACHE["nc"] = _build()
    return _CACHE["nc"]


def _make_in_maps(hidden_states, Wi, Wf, Wg, g_norm_weight, Wo, **_unused):
    bf = ml_dtypes.bfloat16

    def prep_ifg(W):
        # SBUF tile for output block et: w[p, dt*128+e] = W.T[dt*128+p, et*128+e]
        WT = np.ascontiguousarray(np.asarray(W, np.float32).T)
        A = (WT.reshape(DT, 128, ET, 128).transpose(2, 1, 0, 3)
             .reshape(ET * 128, DT * 128))
        return np.ascontiguousarray(A.astype(bf))

    def prep_o(W):
        # SBUF wo[p, (dt*ET+et)*128+d] = W.T[et*128+p, dt*128+d]
        WT = np.ascontiguousarray(np.asarray(W, np.float32).T)
        C = (WT.reshape(ET, 128, DT, 128).transpose(2, 1, 0, 3)
             .reshape(DT * 128, ET * 128))
        return np.ascontiguousarray(C.astype(bf))

    wi = prep_ifg(Wi)
    wf = prep_ifg(Wf)
    wg = prep_ifg(Wg)
    wo = prep_o(Wo)
    gnw = np.ascontiguousarray(
        np.asarray(g_norm_weight, np.float32).reshape(ET, 128).T)
    hs = np.asarray(hidden_states, np.float32)
    in_maps = []
    for c in range(NCORE):
        b, half = c // 2, c % 2
        xt = np.ascontiguousarray(
            hs[b, half * T:(half + 1) * T, :].T).astype(bf)
        mask = np.full((128, 1), float(half), np.float32)
        in_maps.append({"xt": xt, "wi": wi, "wf": wf, "wg": wg,
                        "wo": wo, "gnw": gnw, "mask": mask})
    return in_maps


def kernel(hidden_states, Wi, Wf, Wg, g_norm_weight, Wo, **_unused):
    nc = _get_nc()
    in_maps = _make_in_maps(hidden_states, Wi, Wf, Wg, g_norm_weight, Wo)
    _CACHE["in_maps"] = in_maps
    res = run_bass_kernel_spmd(nc, in_maps, list(range(NCORE))).results
    y = np.empty((B, L, D), np.float32)
    for c in range(NCORE):
        b, half = c // 2, c % 2
        y[b, half * T:(half + 1) * T, :] = res[c]["yt"].T
    return y
